# revision 3
# baseline (speedup 1.0000x reference)
"""DiffPool regression kernel, data-parallel over graphs on 8 NeuronCores.

Contract: kernel(**inputs) takes FULL unsharded inputs (as numpy arrays,
keyed as in setup_inputs()) and returns the FULL [B, 1] float32 output.

Sharding: graphs are sharded 8 ways (1024 graphs/core). Node features,
edge positions are sliced per core on host (pure slicing / layout ops);
all math runs on device via jax.pmap. Weights are replicated.

Hardcoded problem shape (from the spec):
  B=8192 graphs, N=64 nodes/graph, C_IN=128, HID=128, K=5, DEG=8
  total nodes = 524288, E = 4194304, n_cores = 8
"""

import numpy as np
import jax
import jax.numpy as jnp
from functools import partial

B, N, C_IN, HID, K, DEG = 8192, 64, 128, 128, 5, 8
NCORES = 8
GPC = B // NCORES            # graphs per core = 1024
NPC = GPC * N                # nodes per core = 65536
EPG = N * DEG                # edges per graph = 512
EPC = GPC * EPG              # edges per core = 524288


def _fwd(x, srcpos, dstpos, W_pool, b_pool, W1, b1, W2, b2, W_lin, b_lin):
    """Per-core forward. x: [NPC, C_IN] f32; srcpos/dstpos: [GPC, EPG] int32
    node positions within each graph (edges are grouped by graph)."""
    f32 = jnp.float32
    xd = x.reshape(GPC, N, C_IN)                                   # [G,64,128]

    # --- dense adjacency per graph via one-hot matmul (counts, exact) ---
    oh_s = jax.nn.one_hot(srcpos, N, dtype=jnp.bfloat16)            # [G,E,64]
    oh_d = jax.nn.one_hot(dstpos, N, dtype=jnp.bfloat16)            # [G,E,64]
    adj = jnp.einsum('gek,gej->gkj', oh_s, oh_d,
                     preferred_element_type=f32)                    # [G,64,64]

    # --- GCN normalization (PyG DenseGCNConv, add_loop=True) ---
    # diag := 1.0, written without scatter ops (Walrus-friendly)
    eye = jnp.eye(N, dtype=f32)
    adj_l = adj * (1.0 - eye) + eye
    d = jnp.clip(adj_l.sum(-1), 1.0) ** -0.5                        # [G,64]
    adj_n = d[:, :, None] * adj_l * d[:, None, :]

    # One aggregation serves both GCN branches (associativity):
    # adj_n @ (x W) == (adj_n @ x) W
    xa = jnp.einsum('gnm,gmc->gnc', adj_n, xd,
                    preferred_element_type=f32)                     # [G,64,128]
    s_pre = xa @ W_pool + b_pool                                    # [G,64,5]
    x_l1 = xa @ W1 + b1                                             # [G,64,128]

    # mask is all-ones (every graph has exactly N nodes) -> no-op
    s = jax.nn.softmax(s_pre, axis=-1)                              # [G,64,5]
    x_p1 = jnp.einsum('gnk,gnc->gkc', s, x_l1,
                      preferred_element_type=f32)                   # [G,5,128]
    As = jnp.einsum('gnm,gmk->gnk', adj, s,
                    preferred_element_type=f32)                     # [G,64,5]
    adj_p1 = jnp.einsum('gnk,gnl->gkl', s, As,
                        preferred_element_type=f32)                 # [G,5,5]

    # --- second dense GCN on pooled graph (5 nodes) ---
    eye5 = jnp.eye(K, dtype=f32)
    adj2_l = adj_p1 * (1.0 - eye5) + eye5
    d2 = jnp.clip(adj2_l.sum(-1), 1.0) ** -0.5
    adj2_n = d2[:, :, None] * adj2_l * d2[:, None, :]
    h = x_p1 @ W2                                                   # [G,5,128]
    x_l2 = jnp.einsum('gnm,gmc->gnc', adj2_n, h,
                      preferred_element_type=f32) + b2
    pooled = x_l2.sum(axis=1)                                       # [G,128]
    return pooled @ W_lin + b_lin                                   # [G,1]


_PFWD = None


def _get_pfwd():
    global _PFWD
    if _PFWD is None:
        _PFWD = jax.pmap(_fwd, in_axes=(0, 0, 0) + (None,) * 8)
    return _PFWD


def kernel(x, edge_index, batch, W_pool, b_pool, W1, b1, W2, b2, W_lin, b_lin,
           num_graphs, max_nodes):
    x = np.asarray(x, dtype=np.float32)
    ei = np.asarray(edge_index)
    # Node position within graph and per-graph grouping are structural:
    # node i belongs to graph i//N at position i%N; edge e belongs to
    # graph e//(N*DEG). (Host work = slicing + elementwise layout only.)
    srcpos = (ei[0] % N).astype(np.int32).reshape(NCORES, GPC, EPG)
    dstpos = (ei[1] % N).astype(np.int32).reshape(NCORES, GPC, EPG)
    xs = x.reshape(NCORES, NPC, C_IN)

    w = [np.asarray(a, dtype=np.float32)
         for a in (W_pool, b_pool, W1, b1, W2, b2, W_lin, b_lin)]

    out = _get_pfwd()(xs, srcpos, dstpos, *w)                       # [8,GPC,1]
    return np.asarray(out, dtype=np.float32).reshape(B, 1)
